# revision 28
# baseline (speedup 1.0000x reference)
"""Distributed multi-head attention for TRN2 (8 NeuronCores).

Reference computation (per batch b):
    qkv = x @ w_qkv.T                         # (N, 3C)
    q, k, v = split/reshape to (H, N, D)
    attn = softmax(q @ k.T * D**-0.5)         # per head
    out = (attn @ v) reassembled to (N, C)
    out = out @ w_proj.T + b_proj

Sharding: 8 cores = 4 batches x 2 query-halves. Each core computes k/v
for all 2048 tokens of its batch (duplicated across the 2 cores of a
batch - cheaper than communicating), q for its own 1024 tokens, the
full attention for all 12 heads over its 1024 queries, and the output
projection. No collectives.

Layout strategy (all chosen so no on-chip transposes are needed):
  - host passes x^T and w_qkv^T so projections contract over partitions
  - q,k are produced "d-major" ([head-dim, tokens]) via out^T-form
    matmuls; scores are computed transposed ([keys, queries]) which is
    exactly the layout attn@v consumes as its stationary-side operand
  - softmax needs no max-subtraction (scores ~ N(0,1), fp32 exp range)
  - the denominator rides along as a ones-column appended to v (M=65
    matmuls); normalization uses a K=1 ones-matmul to broadcast 1/denom
    across partitions
  - all matmuls in bf16 (PSUM accumulation is fp32); softmax exp runs
    on the scalar (ACT) engine from PSUM f32, writing bf16 probs

Schedule: one flat software pipeline over 96 score-steps (12 passes x
8 k-block pairs), passes hb-alternating so each pair's k/q projection
filler has a 16-step deadline window (per-step PE filler load ~matches
the ACT exp cadence). At pass boundaries the previous pass's ready
attn@v runs before the new pass's scores so the PE never stalls on the
exp double-buffer. The output projection is decomposed into per-pair
partial sums (SBUF f32, DVE adds) emitted as each pair's attnT half is
finalized, leaving only the last pair's term + fused bias-add on the
tail; the final pass's epilogue is pipelined per-head with its drain
copies on the then-idle ACT engine. Input DMAs stay coarse (the
compiler fans big transfers across all 16 queues) in first-consumption
priority order, and the pre-phase is just 2 projection units.

Self-contained: hardcodes B=4, N=2048, C=768, H=12, D=64.
"""

import numpy as np
import ml_dtypes

import concourse.bass as bass
import concourse.mybir as mybir
from concourse import bacc
from concourse.tile import TileContext
from concourse.bass_utils import run_bass_kernel_spmd

F32 = mybir.dt.float32
BF16 = mybir.dt.bfloat16
EXP = mybir.ActivationFunctionType.Exp

B, N, C = 4, 2048, 768
H, D = 12, 64
SCALE = float(D) ** -0.5  # 0.125
NQ = N // 2  # queries per core: 1024
CB = C // 128  # 6 c-chunks
TB = N // 128  # 16 token blocks
HB = H // 2  # 6 head pairs
VW = H * (D + 1)  # 780: v block width with ones columns

N_CORES = 8

# w_qkv columns, grouped in the order the projection units consume them:
# pair-0 k/q, v (split 512+256 for finer DMA deps), then k/q for pairs
# 1..5. Each group holds its column range for all six 128-row input
# chunks, contiguously.
_WQ_GROUPS = [(C, 128), (0, 128), (2 * C, 512), (2 * C + 512, 256)]
for _ob in range(1, CB):
    _WQ_GROUPS.append((C + _ob * 128, 128))
    _WQ_GROUPS.append((_ob * 128, 128))
_WQ_BASE = {}
_cur = 0
for _o0, _w in _WQ_GROUPS:
    _WQ_BASE[_o0] = (_cur, _w)
    _cur += CB * _w

# pass order: hb-alternating keeps the per-pair k/q filler deadlines
# relaxed (pair hb needed by step 16*hb), balancing PE load per step
PASSES = [(hb, qc) for hb in range(HB) for qc in range(2)]
NSTEPS = len(PASSES) * 8  # 96 score-steps, 2 k-blocks each


def _build():
    nc = bacc.Bacc(None, target_bir_lowering=False)

    # host-packed SBUF images: xTp cols = [tch][ci][t]; wqp cols grouped
    # in consumption order (see _WQ_GROUPS)
    xTp = nc.declare_dram_parameter("xTp", [128, CB * N], BF16, isOutput=False)
    wqp = nc.declare_dram_parameter("wqp", [128, CB * 3 * C], BF16, isOutput=False)
    wprojp = nc.declare_dram_parameter("wprojp", [128, CB * C], BF16, isOutput=False)
    biasp = nc.declare_dram_parameter("biasp", [128, CB], F32, isOutput=False)
    outT = nc.declare_dram_parameter("outT", [C, NQ], BF16, isOutput=True)

    with TileContext(nc) as tc:
        with (
            tc.tile_pool(name="per", bufs=1) as per,
            tc.tile_pool(name="p23", bufs=1) as p23,
            tc.tile_pool(name="hp", bufs=8) as hp,
            tc.tile_pool(name="mi", bufs=3) as mi,
            tc.tile_pool(name="op", bufs=2) as op_pool,
            tc.tile_pool(name="wq", bufs=1) as wq_pool,
            tc.tile_pool(name="xt", bufs=1) as xt_pool,
            tc.tile_pool(name="ps", bufs=2, space="PSUM") as ps2,
        ):
            # ---- persistent tiles -------------------------------------
            qT_sb = per.tile([128, CB * NQ], BF16)  # q^T  [2 heads/blk, 1024]
            kT_sb = per.tile([128, CB * N], BF16)  # k^T  [2 heads/blk, 2048]
            vaug_sb = per.tile([128, TB * VW], BF16)  # v + ones cols
            bias_sb = per.tile([128, CB], F32)
            ones_sb = per.tile([1, 64], BF16)
            attnT_sb = p23.tile([128, CB * NQ], BF16)  # attn out^T
            wproj_sb = p23.tile([128, CB * C], BF16)

            nc.vector.memset(ones_sb[:, :], 1.0)
            # ones columns of vaug: col 64 of each 65-wide head slot
            vaug_ones = vaug_sb[:, :].rearrange(
                "p (t h x) -> p t h x", t=TB, h=H, x=D + 1
            )[:, :, :, D : D + 1]
            nc.vector.memset(vaug_ones, 1.0)

            wqkv_sb = wq_pool.tile([128, CB * 3 * C], BF16)
            # coarse per-chunk x tiles: one big DMA each -- the compiler
            # splits large DMAs row-wise across all 16 queues, so coarse
            # transfers load ~10x faster than per-[128,512] chunk DMAs
            xts = [
                xt_pool.tile([128, CB * 512], BF16, name=f"xt{t}")
                for t in range(4)
            ]

            def _dma_xt(tch, split=1):
                w = CB * 512 // split
                for i in range(split):
                    nc.sync.dma_start(
                        out=xts[tch][:, i * w : (i + 1) * w],
                        in_=xTp[:, tch * CB * 512 + i * w : tch * CB * 512 + (i + 1) * w],
                    )

            def _dma_wq(gi, split=1):
                o0, w = _WQ_GROUPS[gi]
                base, _ = _WQ_BASE[o0]
                cw = CB * w // split
                for i in range(split):
                    nc.sync.dma_start(
                        out=wqkv_sb[:, base + i * cw : base + (i + 1) * cw],
                        in_=wqp[:, base + i * cw : base + (i + 1) * cw],
                    )

            # DMA priority order (first-consumption order):
            #   pair0-k + x-chunk0 (pre-phase), pair0-q, x-chunk1,
            #   v weights, x-chunks 2-3, later pairs' k/q, bias/wproj
            _dma_wq(0, split=2)
            _dma_xt(0, split=3)
            _dma_wq(1)
            _dma_xt(1)
            _dma_wq(2)
            _dma_wq(3)
            _dma_xt(2)
            _dma_xt(3)
            for gi in range(4, len(_WQ_GROUPS)):
                _dma_wq(gi)
            nc.sync.dma_start(out=bias_sb[:, :], in_=biasp[:, :])
            nc.sync.dma_start(out=wproj_sb[:, :], in_=wprojp[:, :])

            def wq(ci, o0, width):
                base, gw = _WQ_BASE[o0]
                return wqkv_sb[:, base + ci * gw : base + ci * gw + width]

            # ---- projection work units (PE filler) --------------------
            def kq_unit(ob, tch, is_q):
                """one k^T (or q^T) block: out-dims block ob, 512 tokens"""
                t0 = tch * 512
                kind = "q" if is_q else "k"
                psv = ps2.tile(
                    [128, 512], F32, tag="psV", bufs=2, name=f"{kind}{ob}_{tch}"
                )
                for ci in range(CB):
                    nc.tensor.matmul(
                        psv[:, :],
                        wq(ci, (0 if is_q else C) + ob * 128, 128),
                        xts[tch][:, ci * 512 : (ci + 1) * 512],
                        start=(ci == 0),
                        stop=(ci == CB - 1),
                    )
                if is_q:
                    nc.vector.tensor_copy(
                        qT_sb[:, ob * NQ + t0 : ob * NQ + t0 + 512], psv[:, :]
                    )
                else:
                    nc.vector.tensor_copy(
                        kT_sb[:, ob * N + t0 : ob * N + t0 + 512], psv[:, :]
                    )

            def v_unit(t128, o0, w):
                """one v unit: 128 tokens x [o0, o0+w) v-dims, written
                (bf16) into the vaug slot layout"""
                tch, tb = divmod(t128, 4)
                psv = ps2.tile(
                    [128, 512], F32, tag="psV", bufs=2, name=f"v{t128}_{o0}"
                )
                for ci in range(CB):
                    nc.tensor.matmul(
                        psv[:, :w],
                        xts[tch][:, ci * 512 + tb * 128 : ci * 512 + (tb + 1) * 128],
                        wq(ci, 2 * C + o0, w),
                        start=(ci == 0),
                        stop=(ci == CB - 1),
                    )
                nh = w // D
                src = psv[:, :w].rearrange("p (h x) -> p h x", x=D)
                h0 = o0 // D
                base = t128 * VW + h0 * (D + 1)
                dst = vaug_sb[:, base : base + nh * (D + 1)].rearrange(
                    "p (h x) -> p h x", x=D + 1
                )[:, :, :D]
                nc.vector.tensor_copy(dst, src)

            def proj_unit(ob, qc):
                """output projection for out-dims block ob, query half qc
                (contracts all 6 attnT pair-blocks), bias add + DMA out"""
                psp = ps2.tile(
                    [128, 512], F32, tag="psV", bufs=2, name=f"prj{ob}_{qc}"
                )
                for cb in range(CB):
                    nc.tensor.matmul(
                        psp[:, :],
                        wproj_sb[:, cb * C + ob * 128 : cb * C + (ob + 1) * 128],
                        attnT_sb[:, cb * NQ + qc * 512 : cb * NQ + (qc + 1) * 512],
                        start=(cb == 0),
                        stop=(cb == CB - 1),
                    )
                ot = op_pool.tile([128, 512], BF16, tag="out")
                nc.vector.tensor_scalar_add(
                    ot[:, :], psp[:, :], bias_sb[:, ob : ob + 1]
                )
                nc.sync.dma_start(
                    out=outT[ob * 128 : (ob + 1) * 128, qc * 512 : (qc + 1) * 512],
                    in_=ot[:, :],
                )

            # output projection as per-pair partial sums: psq[qc][ob]
            # accumulates wproj_cb^T @ attnT[cb] in SBUF f32 (DVE adds) as
            # each pair's attnT half becomes final; only the last pair's
            # term + bias is on the tail/late critical path
            projp_sb = [
                [p23.tile([128, 512], F32, name=f"pp{qc}_{ob}") for ob in range(CB)]
                for qc in range(2)
            ]

            def proj_partial(cb, ob, qc):
                psp = ps2.tile(
                    [128, 512], F32, tag="psV", bufs=2, name=f"pj{cb}_{ob}_{qc}"
                )
                nc.tensor.matmul(
                    psp[:, :],
                    wproj_sb[:, cb * C + ob * 128 : cb * C + (ob + 1) * 128],
                    attnT_sb[:, cb * NQ + qc * 512 : cb * NQ + (qc + 1) * 512],
                    start=True,
                    stop=True,
                )
                if cb == 0:
                    nc.vector.tensor_copy(projp_sb[qc][ob][:, :], psp[:, :])
                else:
                    nc.vector.tensor_add(
                        projp_sb[qc][ob][:, :], projp_sb[qc][ob][:, :], psp[:, :]
                    )

            def proj_final(ob, qc):
                cb = CB - 1
                psp = ps2.tile(
                    [128, 512], F32, tag="psV", bufs=2, name=f"pjf{ob}_{qc}"
                )
                nc.tensor.matmul(
                    psp[:, :],
                    wproj_sb[:, cb * C + ob * 128 : cb * C + (ob + 1) * 128],
                    attnT_sb[:, cb * NQ + qc * 512 : cb * NQ + (qc + 1) * 512],
                    start=True,
                    stop=True,
                )
                ot = op_pool.tile([128, 512], BF16, tag="out")
                # (psp + bias) + partial, fused on DVE
                nc.vector.scalar_tensor_tensor(
                    ot[:, :],
                    psp[:, :],
                    bias_sb[:, ob : ob + 1],
                    projp_sb[qc][ob][:, :],
                    op0=mybir.AluOpType.add,
                    op1=mybir.AluOpType.add,
                )
                nc.sync.dma_start(
                    out=outT[ob * 128 : (ob + 1) * 128, qc * 512 : (qc + 1) * 512],
                    in_=ot[:, :],
                )

            # ---- filler schedule --------------------------------------
            # filler[t] = list of closures to emit in step t's filler slot
            filler = [[] for _ in range(NSTEPS)]

            def _sched(t, fn):
                filler[t].append(fn)

            # remaining pair-0 k chunks first (their x chunks land before
            # the v weights): k tch needed by scores kb=4*tch (step 2*tch)
            for tch in range(1, 4):
                _sched(tch - 1, (lambda t=tch: kq_unit(0, t, False)))
            _sched(4, (lambda: kq_unit(0, 1, True)))  # q tch1 by step 8
            # v blocks just-in-time for pass 0: block kb in step kb//2
            for kb in range(TB):
                _sched(kb // 2, (lambda kb=kb: v_unit(kb, 0, 512)))
                _sched(kb // 2, (lambda kb=kb: v_unit(kb, 512, 256)))
            # pairs 1-5: k tch j needed by step 16*hb + 2*j, q tch0 by
            # 16*hb, q tch1 by 16*hb + 8; weights arrive ~step 5-8
            for hb in range(1, HB):
                for j in range(4):
                    _sched(16 * hb - 9 + 2 * j,
                           (lambda h=hb, j=j: kq_unit(h, j, False)))
                _sched(16 * hb - 2, (lambda h=hb: kq_unit(h, 0, True)))
                _sched(16 * hb + 4, (lambda h=hb: kq_unit(h, 1, True)))
            # out-proj partials: pair cb's (qc) half is final after pass
            # idx 2*cb+qc's epilogue (~step 16*cb+8*qc+10)
            for cb in range(CB - 1):
                for ob in range(CB):
                    _sched(16 * cb + 11 + ob,
                           (lambda c=cb, o=ob: proj_partial(c, o, 0)))
                    _sched(16 * cb + 19 + ob,
                           (lambda c=cb, o=ob: proj_partial(c, o, 1)))
            # qc=0 finals are emitted in the tail (after the last avs)
            # so they execute during the final epilogue's DVE/ACT waits

            # ---- attention pipeline -----------------------------------
            def epi_pe(hb_, qc_, outs_):
                """PE part of a pass's normalization epilogue. The two
                heads' 1/denom broadcasts go to different column strips of
                one PSUM tile (col tiling) so they run concurrently."""
                psb = ps2.tile(
                    [128, 512], F32, tag="psV", bufs=2,
                    name=f"psb{hb_}_{qc_}",
                )
                for hh_ in range(2):
                    nc.tensor.matmul(
                        psb[64 * hh_ : 64 * hh_ + 64, :],
                        ones_sb[:, :],
                        outs_[hh_][1][:, :],
                        start=True,
                        stop=True,
                    )
                for hh_ in range(2):
                    nc.vector.tensor_mul(
                        attnT_sb[
                            64 * hh_ : 64 * hh_ + 64,
                            hb_ * NQ + qc_ * 512 : hb_ * NQ + (qc_ + 1) * 512,
                        ],
                        psb[64 * hh_ : 64 * hh_ + 64, :],
                        outs_[hh_][0][:, :],
                    )

            def emit_scores(hb, qc, kb2):
                """scores for k-blocks kb2, kb2+1 (both heads) -> psS
                tiles + exp -> probs; returns [(kb, srcs), ...]"""
                q0 = hb * NQ + qc * 512
                out = []
                for kb in (kb2, kb2 + 1):
                    sc = ps2.tile(
                        [128, NQ], F32, tag="psS", bufs=2,
                        name=f"sc{hb}_{qc}_{kb}",
                    )
                    for hh in range(2):
                        p0 = 64 * hh
                        nc.tensor.matmul(
                            sc[:, hh * 512 : (hh + 1) * 512],
                            kT_sb[
                                p0 : p0 + 64,
                                hb * N + kb * 128 : hb * N + (kb + 1) * 128,
                            ],
                            qT_sb[p0 : p0 + 64, q0 : q0 + 512],
                            start=True,
                            stop=True,
                            tile_position=(p0, 0),
                        )
                    out.append((kb, [(sc, 0, NQ)]))
                return out

            def emit_exps(pending_sc):
                out = []
                for kb, srcs in pending_sc:
                    pb = hp.tile([128, NQ], BF16, tag="probs")
                    for src, col, w in srcs:
                        nc.scalar.activation(
                            pb[:, col : col + w], src[:, :w], EXP, scale=SCALE
                        )
                    out.append((kb, pb))
                return out

            def av_mms(accs, hb, pkb, ppb, heads=(0, 1)):
                for hh in heads:
                    vs = pkb * VW + (2 * hb + hh) * (D + 1)
                    nc.tensor.matmul(
                        accs[hh][0:65, :],
                        vaug_sb[:, vs : vs + D + 1],
                        ppb[:, hh * 512 : (hh + 1) * 512],
                        start=(pkb == 0),
                        stop=(pkb == TB - 1),
                    )

            def drain(accs):
                """epilogue: drain accumulators (on the otherwise-idle
                Pool engine) + 1/denominator (DVE)"""
                outs = []
                for hh in range(2):
                    acc = accs[hh]
                    cpy = mi.tile([64, 512], F32, tag="cpy")
                    nc.vector.tensor_copy(cpy[:, :], acc[0:64, :])
                    den = mi.tile([1, 512], F32, tag="den")
                    nc.vector.tensor_copy(den[:, :], acc[64:65, :])
                    rec = mi.tile([1, 512], F32, tag="rec")
                    nc.vector.reciprocal_approx_fast(rec[:, :], den[:, :])
                    row = mi.tile([1, 512], BF16, tag="row")
                    nc.vector.tensor_copy(row[:, :], rec[:, :])
                    outs.append((cpy, row))
                return outs

            # ---- pre-phase: minimal (pair-0 k/q for the first chunk) --
            kq_unit(0, 0, False)
            kq_unit(0, 0, True)

            # ---- the flat pipeline ------------------------------------
            accs = None  # current pass's psA accumulators
            prev_probs = []  # [(kb, probs)] awaiting attn@v
            prev_pass = None  # (hb, qc) owning prev_probs
            pend_epi = None  # (hb, qc, outs, due_step)

            def new_accs(hb, qc):
                return [
                    ps2.tile(
                        [128, 512], F32, tag="psA", bufs=2,
                        name=f"acc{hb}_{qc}_{i}",
                    )
                    for i in range(2)
                ]

            for t in range(NSTEPS):
                hb, qc = PASSES[t // 8]
                kb2 = (t % 8) * 2
                # mid-pass: scores first (keeps ACT fed); at boundaries the
                # new pass's scores wait on the exp pipeline, so run the
                # previous pass's ready attn@v first to keep the PE busy
                if kb2 == 0 and prev_probs:
                    for pkb, ppb in prev_probs:
                        av_mms(accs, prev_pass[0], pkb, ppb)
                    pend_sc = emit_scores(hb, qc, kb2)
                else:
                    pend_sc = emit_scores(hb, qc, kb2)
                    for pkb, ppb in prev_probs:
                        av_mms(accs, prev_pass[0], pkb, ppb)
                if prev_probs and prev_probs[-1][0] == TB - 1:
                    # previous pass complete: drain + defer its epilogue,
                    # then recycle the acc slots for the new pass
                    outs = drain(accs)
                    pend_epi = (prev_pass[0], prev_pass[1], outs, t + 1)
                    accs = new_accs(hb, qc)
                elif accs is None:
                    accs = new_accs(hb, qc)
                # filler work
                for fn in filler[t]:
                    fn()
                # deferred epilogue of the pass before last
                if pend_epi is not None and t >= pend_epi[3]:
                    epi_pe(pend_epi[0], pend_epi[1], pend_epi[2])
                    pend_epi = None
                # exp of this step's scores
                prev_probs = emit_exps(pend_sc)
                prev_pass = (hb, qc)

            # ---- tail: last pass's attn@v drain + epilogue ------------
            if pend_epi is not None:
                epi_pe(pend_epi[0], pend_epi[1], pend_epi[2])
                pend_epi = None
            # head-major: head 0's drain/normalize chain (ACT copies,
            # DVE recip, broadcast matmul, multiply) runs under head 1's
            # attn@v stream
            hb_l, qc_l = prev_pass
            psb_l = ps2.tile([128, 512], F32, tag="psV", bufs=2, name="psbL")
            for hh in range(2):
                for pkb, ppb in prev_probs:
                    av_mms(accs, prev_pass[0], pkb, ppb, heads=(hh,))
                acc = accs[hh]
                den = mi.tile([1, 512], F32, tag="den")
                nc.scalar.copy(den[:, :], acc[64:65, :])
                rec = mi.tile([1, 512], F32, tag="rec")
                nc.vector.reciprocal_approx_fast(rec[:, :], den[:, :])
                row = mi.tile([1, 512], BF16, tag="row")
                nc.vector.tensor_copy(row[:, :], rec[:, :])
                cpy = mi.tile([64, 512], F32, tag="cpy")
                nc.scalar.copy(cpy[:, :], acc[0:64, :])
                nc.tensor.matmul(
                    psb_l[64 * hh : 64 * hh + 64, :],
                    ones_sb[:, :],
                    row[:, :],
                    start=True,
                    stop=True,
                )
                nc.vector.tensor_mul(
                    attnT_sb[
                        64 * hh : 64 * hh + 64,
                        hb_l * NQ + qc_l * 512 : hb_l * NQ + (qc_l + 1) * 512,
                    ],
                    psb_l[64 * hh : 64 * hh + 64, :],
                    cpy[:, :],
                )

            # ---- output projection finals -----------------------------
            # qc=0 first (independent of the last pass's epilogue): these
            # matmuls fill the PE waits under the DVE normalize chain
            for ob in range(CB):
                proj_final(ob, 0)
            for ob in range(CB):
                proj_final(ob, 1)

    nc.finalize()
    return nc


_NC_CACHE = []


def _get_nc():
    if not _NC_CACHE:
        _NC_CACHE.append(_build())
    return _NC_CACHE[0]


def kernel(x, w_qkv, w_proj, b_proj):
    x = np.asarray(x, dtype=np.float32)
    w_qkv = np.asarray(w_qkv, dtype=np.float32)
    w_proj = np.asarray(w_proj, dtype=np.float32)
    b_proj = np.asarray(b_proj, dtype=np.float32)

    nc = _get_nc()

    wqkvT = w_qkv.T.astype(ml_dtypes.bfloat16)  # [C, 3C]
    wq3 = np.ascontiguousarray(wqkvT).reshape(CB, 128, 3 * C)  # [ci, p, o]
    wqp = np.concatenate(
        [
            wq3[:, :, o0 : o0 + w].transpose(1, 0, 2).reshape(128, CB * w)
            for o0, w in _WQ_GROUPS
        ],
        axis=1,
    )
    wqp = np.ascontiguousarray(wqp)
    # SBUF images: wproj cols = [ci][o], bias cols = [ci]
    wprojp = np.ascontiguousarray(
        w_proj.T.astype(ml_dtypes.bfloat16).reshape(CB, 128, C)
        .transpose(1, 0, 2)
        .reshape(128, CB * C)
    )
    biasp = np.ascontiguousarray(
        b_proj.astype(np.float32).reshape(CB, 128).T
    )

    in_maps = []
    for core in range(N_CORES):
        b, half = divmod(core, 2)
        # own 1024 query tokens first, then the other half (key order
        # within attention is permutation-invariant)
        mine = x[b, half * NQ : (half + 1) * NQ].T
        other = x[b, (1 - half) * NQ : (2 - half) * NQ].T
        xTc = np.concatenate([mine, other], axis=1).astype(ml_dtypes.bfloat16)
        # pack to the SBUF image: cols = [tch][ci][t]
        xTp = np.ascontiguousarray(
            xTc.reshape(CB, 128, 4, 512).transpose(1, 2, 0, 3).reshape(128, CB * N)
        )
        in_maps.append({"xTp": xTp, "wqp": wqp, "wprojp": wprojp, "biasp": biasp})

    res = run_bass_kernel_spmd(nc, in_maps, core_ids=list(range(N_CORES)))

    out = np.empty((B, N, C), dtype=np.float32)
    for core in range(N_CORES):
        b, half = divmod(core, 2)
        out[b, half * NQ : (half + 1) * NQ, :] = (
            res.results[core]["outT"].astype(np.float32).T
        )
    return out


# revision 29
# speedup vs baseline: 1.0227x; 1.0227x over previous
"""Distributed multi-head attention for TRN2 (8 NeuronCores).

Reference computation (per batch b):
    qkv = x @ w_qkv.T                         # (N, 3C)
    q, k, v = split/reshape to (H, N, D)
    attn = softmax(q @ k.T * D**-0.5)         # per head
    out = (attn @ v) reassembled to (N, C)
    out = out @ w_proj.T + b_proj

Sharding: 8 cores = 4 batches x 2 query-halves. Each core computes k/v
for all 2048 tokens of its batch (duplicated across the 2 cores of a
batch - cheaper than communicating), q for its own 1024 tokens, the
full attention for all 12 heads over its 1024 queries, and the output
projection. No collectives.

Layout strategy (all chosen so no on-chip transposes are needed):
  - host passes x^T and w_qkv^T so projections contract over partitions
  - q,k are produced "d-major" ([head-dim, tokens]) via out^T-form
    matmuls; scores are computed transposed ([keys, queries]) which is
    exactly the layout attn@v consumes as its stationary-side operand
  - softmax needs no max-subtraction (scores ~ N(0,1), fp32 exp range)
  - the denominator rides along as a ones-column appended to v (M=65
    matmuls); normalization uses a K=1 ones-matmul to broadcast 1/denom
    across partitions
  - all matmuls in bf16 (PSUM accumulation is fp32); softmax exp runs
    on the scalar (ACT) engine from PSUM f32, writing bf16 probs

Schedule: one flat software pipeline over 96 score-steps (12 passes x
8 k-block pairs), passes hb-alternating so each pair's k/q projection
filler has a 16-step deadline window (per-step PE filler load ~matches
the ACT exp cadence). At pass boundaries the previous pass's ready
attn@v runs before the new pass's scores so the PE never stalls on the
exp double-buffer. The output projection is decomposed into per-pair
partial sums (SBUF f32, DVE adds) emitted as each pair's attnT half is
finalized, leaving only the last pair's term + fused bias-add on the
tail; the final pass's epilogue is pipelined per-head with its drain
copies on the then-idle ACT engine. Input DMAs stay coarse (the
compiler fans big transfers across all 16 queues) in first-consumption
priority order, and the pre-phase is just 2 projection units.

Self-contained: hardcodes B=4, N=2048, C=768, H=12, D=64.
"""

import numpy as np
import ml_dtypes

import concourse.bass as bass
import concourse.mybir as mybir
from concourse import bacc
from concourse.tile import TileContext
from concourse.bass_utils import run_bass_kernel_spmd

F32 = mybir.dt.float32
BF16 = mybir.dt.bfloat16
EXP = mybir.ActivationFunctionType.Exp

B, N, C = 4, 2048, 768
H, D = 12, 64
SCALE = float(D) ** -0.5  # 0.125
NQ = N // 2  # queries per core: 1024
CB = C // 128  # 6 c-chunks
TB = N // 128  # 16 token blocks
HB = H // 2  # 6 head pairs
VW = H * (D + 1)  # 780: v block width with ones columns

N_CORES = 8

# w_qkv columns, grouped in the order the projection units consume them:
# pair-0 k/q, v (split 512+256 for finer DMA deps), then k/q for pairs
# 1..5. Each group holds its column range for all six 128-row input
# chunks, contiguously.
_WQ_GROUPS = [(C, 128), (0, 128), (2 * C, 512), (2 * C + 512, 256)]
for _ob in range(1, CB):
    _WQ_GROUPS.append((C + _ob * 128, 128))
    _WQ_GROUPS.append((_ob * 128, 128))
_WQ_BASE = {}
_cur = 0
for _o0, _w in _WQ_GROUPS:
    _WQ_BASE[_o0] = (_cur, _w)
    _cur += CB * _w

# pass order: hb-alternating keeps the per-pair k/q filler deadlines
# relaxed (pair hb needed by step 16*hb), balancing PE load per step
PASSES = [(hb, qc) for hb in range(HB) for qc in range(2)]
NSTEPS = len(PASSES) * 8  # 96 score-steps, 2 k-blocks each


def _build():
    nc = bacc.Bacc(None, target_bir_lowering=False)

    # host-packed SBUF images: xTp cols = [tch][ci][t]; wqp cols grouped
    # in consumption order (see _WQ_GROUPS)
    xTp = nc.declare_dram_parameter("xTp", [128, CB * N], BF16, isOutput=False)
    wqp = nc.declare_dram_parameter("wqp", [128, CB * 3 * C], BF16, isOutput=False)
    wprojp = nc.declare_dram_parameter("wprojp", [128, CB * C], BF16, isOutput=False)
    biasp = nc.declare_dram_parameter("biasp", [128, CB], F32, isOutput=False)
    outT = nc.declare_dram_parameter("outT", [C, NQ], BF16, isOutput=True)

    with TileContext(nc) as tc:
        with (
            tc.tile_pool(name="per", bufs=1) as per,
            tc.tile_pool(name="p23", bufs=1) as p23,
            tc.tile_pool(name="hp", bufs=8) as hp,
            tc.tile_pool(name="mi", bufs=3) as mi,
            tc.tile_pool(name="op", bufs=2) as op_pool,
            tc.tile_pool(name="wq", bufs=1) as wq_pool,
            tc.tile_pool(name="xt", bufs=1) as xt_pool,
            tc.tile_pool(name="ps", bufs=2, space="PSUM") as ps2,
        ):
            # ---- persistent tiles -------------------------------------
            qT_sb = per.tile([128, CB * NQ], BF16)  # q^T  [2 heads/blk, 1024]
            kT_sb = per.tile([128, CB * N], BF16)  # k^T  [2 heads/blk, 2048]
            vaug_sb = per.tile([128, TB * VW], BF16)  # v + ones cols
            bias_sb = per.tile([128, CB], F32)
            ones_sb = per.tile([1, 64], BF16)
            attnT_sb = p23.tile([128, CB * NQ], BF16)  # attn out^T
            wproj_sb = p23.tile([128, CB * C], BF16)

            nc.vector.memset(ones_sb[:, :], 1.0)
            # ones columns of vaug: col 64 of each 65-wide head slot
            vaug_ones = vaug_sb[:, :].rearrange(
                "p (t h x) -> p t h x", t=TB, h=H, x=D + 1
            )[:, :, :, D : D + 1]
            nc.vector.memset(vaug_ones, 1.0)

            wqkv_sb = wq_pool.tile([128, CB * 3 * C], BF16)
            # coarse per-chunk x tiles: one big DMA each -- the compiler
            # splits large DMAs row-wise across all 16 queues, so coarse
            # transfers load ~10x faster than per-[128,512] chunk DMAs
            xts = [
                xt_pool.tile([128, CB * 512], BF16, name=f"xt{t}")
                for t in range(4)
            ]

            def _dma_xt(tch, split=1):
                w = CB * 512 // split
                for i in range(split):
                    nc.sync.dma_start(
                        out=xts[tch][:, i * w : (i + 1) * w],
                        in_=xTp[:, tch * CB * 512 + i * w : tch * CB * 512 + (i + 1) * w],
                    )

            def _dma_wq(gi, split=1):
                o0, w = _WQ_GROUPS[gi]
                base, _ = _WQ_BASE[o0]
                cw = CB * w // split
                for i in range(split):
                    nc.sync.dma_start(
                        out=wqkv_sb[:, base + i * cw : base + (i + 1) * cw],
                        in_=wqp[:, base + i * cw : base + (i + 1) * cw],
                    )

            # DMA priority order (first-consumption order):
            #   pair0-k + x-chunk0 (pre-phase), pair0-q, x-chunk1,
            #   v weights, x-chunks 2-3, later pairs' k/q, bias/wproj
            _dma_wq(0, split=2)
            _dma_xt(0, split=3)
            _dma_wq(1)
            _dma_xt(1)
            _dma_wq(2)
            _dma_wq(3)
            _dma_xt(2)
            _dma_xt(3)
            for gi in range(4, len(_WQ_GROUPS)):
                _dma_wq(gi)
            nc.sync.dma_start(out=bias_sb[:, :], in_=biasp[:, :])
            nc.sync.dma_start(out=wproj_sb[:, :], in_=wprojp[:, :])

            def wq(ci, o0, width):
                base, gw = _WQ_BASE[o0]
                return wqkv_sb[:, base + ci * gw : base + ci * gw + width]

            # ---- projection work units (PE filler) --------------------
            def kq_unit(ob, tch, is_q):
                """one k^T (or q^T) block: out-dims block ob, 512 tokens"""
                t0 = tch * 512
                kind = "q" if is_q else "k"
                psv = ps2.tile(
                    [128, 512], F32, tag="psV", bufs=2, name=f"{kind}{ob}_{tch}"
                )
                for ci in range(CB):
                    nc.tensor.matmul(
                        psv[:, :],
                        wq(ci, (0 if is_q else C) + ob * 128, 128),
                        xts[tch][:, ci * 512 : (ci + 1) * 512],
                        start=(ci == 0),
                        stop=(ci == CB - 1),
                    )
                if is_q:
                    nc.vector.tensor_copy(
                        qT_sb[:, ob * NQ + t0 : ob * NQ + t0 + 512], psv[:, :]
                    )
                else:
                    nc.vector.tensor_copy(
                        kT_sb[:, ob * N + t0 : ob * N + t0 + 512], psv[:, :]
                    )

            def v_unit(t128, o0, w):
                """one v unit: 128 tokens x [o0, o0+w) v-dims, written
                (bf16) into the vaug slot layout"""
                tch, tb = divmod(t128, 4)
                psv = ps2.tile(
                    [128, 512], F32, tag="psV", bufs=2, name=f"v{t128}_{o0}"
                )
                for ci in range(CB):
                    nc.tensor.matmul(
                        psv[:, :w],
                        xts[tch][:, ci * 512 + tb * 128 : ci * 512 + (tb + 1) * 128],
                        wq(ci, 2 * C + o0, w),
                        start=(ci == 0),
                        stop=(ci == CB - 1),
                    )
                nh = w // D
                src = psv[:, :w].rearrange("p (h x) -> p h x", x=D)
                h0 = o0 // D
                base = t128 * VW + h0 * (D + 1)
                dst = vaug_sb[:, base : base + nh * (D + 1)].rearrange(
                    "p (h x) -> p h x", x=D + 1
                )[:, :, :D]
                nc.vector.tensor_copy(dst, src)

            def proj_unit(ob, qc):
                """output projection for out-dims block ob, query half qc
                (contracts all 6 attnT pair-blocks), bias add + DMA out"""
                psp = ps2.tile(
                    [128, 512], F32, tag="psV", bufs=2, name=f"prj{ob}_{qc}"
                )
                for cb in range(CB):
                    nc.tensor.matmul(
                        psp[:, :],
                        wproj_sb[:, cb * C + ob * 128 : cb * C + (ob + 1) * 128],
                        attnT_sb[:, cb * NQ + qc * 512 : cb * NQ + (qc + 1) * 512],
                        start=(cb == 0),
                        stop=(cb == CB - 1),
                    )
                ot = op_pool.tile([128, 512], BF16, tag="out")
                nc.vector.tensor_scalar_add(
                    ot[:, :], psp[:, :], bias_sb[:, ob : ob + 1]
                )
                nc.sync.dma_start(
                    out=outT[ob * 128 : (ob + 1) * 128, qc * 512 : (qc + 1) * 512],
                    in_=ot[:, :],
                )

            # output projection as per-pair partial sums: psq[qc][ob]
            # accumulates wproj_cb^T @ attnT[cb] in SBUF f32 (DVE adds) as
            # each pair's attnT half becomes final; only the last pair's
            # term + bias is on the tail/late critical path
            projp_sb = [
                [p23.tile([128, 512], F32, name=f"pp{qc}_{ob}") for ob in range(CB)]
                for qc in range(2)
            ]

            def proj_partial(cb, ob, qc):
                psp = ps2.tile(
                    [128, 512], F32, tag="psV", bufs=2, name=f"pj{cb}_{ob}_{qc}"
                )
                nc.tensor.matmul(
                    psp[:, :],
                    wproj_sb[:, cb * C + ob * 128 : cb * C + (ob + 1) * 128],
                    attnT_sb[:, cb * NQ + qc * 512 : cb * NQ + (qc + 1) * 512],
                    start=True,
                    stop=True,
                )
                if cb == 0:
                    nc.vector.tensor_copy(projp_sb[qc][ob][:, :], psp[:, :])
                else:
                    nc.vector.tensor_add(
                        projp_sb[qc][ob][:, :], projp_sb[qc][ob][:, :], psp[:, :]
                    )

            def proj_final(ob, qc):
                cb = CB - 1
                psp = ps2.tile(
                    [128, 512], F32, tag="psV", bufs=2, name=f"pjf{ob}_{qc}"
                )
                nc.tensor.matmul(
                    psp[:, :],
                    wproj_sb[:, cb * C + ob * 128 : cb * C + (ob + 1) * 128],
                    attnT_sb[:, cb * NQ + qc * 512 : cb * NQ + (qc + 1) * 512],
                    start=True,
                    stop=True,
                )
                ot = op_pool.tile([128, 512], BF16, tag="out")
                # (psp + bias) + partial, fused on DVE
                nc.vector.scalar_tensor_tensor(
                    ot[:, :],
                    psp[:, :],
                    bias_sb[:, ob : ob + 1],
                    projp_sb[qc][ob][:, :],
                    op0=mybir.AluOpType.add,
                    op1=mybir.AluOpType.add,
                )
                nc.sync.dma_start(
                    out=outT[ob * 128 : (ob + 1) * 128, qc * 512 : (qc + 1) * 512],
                    in_=ot[:, :],
                )

            # ---- filler schedule --------------------------------------
            # filler[t] = list of closures to emit in step t's filler slot
            filler = [[] for _ in range(NSTEPS)]

            def _sched(t, fn):
                filler[t].append(fn)

            # remaining pair-0 k chunks first (their x chunks land before
            # the v weights): k tch needed by scores kb=4*tch (step 2*tch)
            for tch in range(1, 4):
                _sched(tch - 1, (lambda t=tch: kq_unit(0, t, False)))
            _sched(4, (lambda: kq_unit(0, 1, True)))  # q tch1 by step 8
            # v blocks just-in-time for pass 0: block kb in step kb//2
            for kb in range(TB):
                _sched(kb // 2, (lambda kb=kb: v_unit(kb, 0, 512)))
                _sched(kb // 2, (lambda kb=kb: v_unit(kb, 512, 256)))
            # pairs 1-5: k tch j needed by step 16*hb + 2*j, q tch0 by
            # 16*hb, q tch1 by 16*hb + 8; weights arrive ~step 5-8
            for hb in range(1, HB):
                for j in range(4):
                    _sched(16 * hb - 9 + 2 * j,
                           (lambda h=hb, j=j: kq_unit(h, j, False)))
                _sched(16 * hb - 2, (lambda h=hb: kq_unit(h, 0, True)))
                _sched(16 * hb + 4, (lambda h=hb: kq_unit(h, 1, True)))
            # out-proj partials: pair cb's (qc) half is final after pass
            # idx 2*cb+qc's epilogue (~step 16*cb+8*qc+10)
            for cb in range(CB - 1):
                for ob in range(CB):
                    _sched(16 * cb + 11 + ob,
                           (lambda c=cb, o=ob: proj_partial(c, o, 0)))
                    _sched(16 * cb + 19 + ob,
                           (lambda c=cb, o=ob: proj_partial(c, o, 1)))
            # qc=0 finals: pair-5 qc0 epilogue fires at step 89
            for ob in range(CB):
                _sched(90 + ob, (lambda o=ob: proj_final(o, 0)))

            # ---- attention pipeline -----------------------------------
            def epi_pe(hb_, qc_, outs_):
                """PE part of a pass's normalization epilogue. The two
                heads' 1/denom broadcasts go to different column strips of
                one PSUM tile (col tiling) so they run concurrently."""
                psb = ps2.tile(
                    [128, 512], F32, tag="psV", bufs=2,
                    name=f"psb{hb_}_{qc_}",
                )
                for hh_ in range(2):
                    nc.tensor.matmul(
                        psb[64 * hh_ : 64 * hh_ + 64, :],
                        ones_sb[:, :],
                        outs_[hh_][1][:, :],
                        start=True,
                        stop=True,
                    )
                for hh_ in range(2):
                    nc.vector.tensor_mul(
                        attnT_sb[
                            64 * hh_ : 64 * hh_ + 64,
                            hb_ * NQ + qc_ * 512 : hb_ * NQ + (qc_ + 1) * 512,
                        ],
                        psb[64 * hh_ : 64 * hh_ + 64, :],
                        outs_[hh_][0][:, :],
                    )

            def emit_scores(hb, qc, kb2):
                """scores for k-blocks kb2, kb2+1 (both heads) -> psS
                tiles + exp -> probs; returns [(kb, srcs), ...]"""
                q0 = hb * NQ + qc * 512
                out = []
                for kb in (kb2, kb2 + 1):
                    sc = ps2.tile(
                        [128, NQ], F32, tag="psS", bufs=2,
                        name=f"sc{hb}_{qc}_{kb}",
                    )
                    for hh in range(2):
                        p0 = 64 * hh
                        nc.tensor.matmul(
                            sc[:, hh * 512 : (hh + 1) * 512],
                            kT_sb[
                                p0 : p0 + 64,
                                hb * N + kb * 128 : hb * N + (kb + 1) * 128,
                            ],
                            qT_sb[p0 : p0 + 64, q0 : q0 + 512],
                            start=True,
                            stop=True,
                            tile_position=(p0, 0),
                        )
                    out.append((kb, [(sc, 0, NQ)]))
                return out

            def emit_exps(pending_sc):
                out = []
                for kb, srcs in pending_sc:
                    pb = hp.tile([128, NQ], BF16, tag="probs")
                    for src, col, w in srcs:
                        nc.scalar.activation(
                            pb[:, col : col + w], src[:, :w], EXP, scale=SCALE
                        )
                    out.append((kb, pb))
                return out

            def av_mms(accs, hb, pkb, ppb, heads=(0, 1)):
                for hh in heads:
                    vs = pkb * VW + (2 * hb + hh) * (D + 1)
                    nc.tensor.matmul(
                        accs[hh][0:65, :],
                        vaug_sb[:, vs : vs + D + 1],
                        ppb[:, hh * 512 : (hh + 1) * 512],
                        start=(pkb == 0),
                        stop=(pkb == TB - 1),
                    )

            def drain(accs):
                """epilogue: drain accumulators (on the otherwise-idle
                Pool engine) + 1/denominator (DVE)"""
                outs = []
                for hh in range(2):
                    acc = accs[hh]
                    cpy = mi.tile([64, 512], F32, tag="cpy")
                    nc.vector.tensor_copy(cpy[:, :], acc[0:64, :])
                    den = mi.tile([1, 512], F32, tag="den")
                    nc.vector.tensor_copy(den[:, :], acc[64:65, :])
                    rec = mi.tile([1, 512], F32, tag="rec")
                    nc.vector.reciprocal_approx_fast(rec[:, :], den[:, :])
                    row = mi.tile([1, 512], BF16, tag="row")
                    nc.vector.tensor_copy(row[:, :], rec[:, :])
                    outs.append((cpy, row))
                return outs

            # ---- pre-phase: minimal (pair-0 k/q for the first chunk) --
            kq_unit(0, 0, False)
            kq_unit(0, 0, True)

            # ---- the flat pipeline ------------------------------------
            accs = None  # current pass's psA accumulators
            prev_probs = []  # [(kb, probs)] awaiting attn@v
            prev_pass = None  # (hb, qc) owning prev_probs
            pend_epi = None  # (hb, qc, outs, due_step)

            def new_accs(hb, qc):
                return [
                    ps2.tile(
                        [128, 512], F32, tag="psA", bufs=2,
                        name=f"acc{hb}_{qc}_{i}",
                    )
                    for i in range(2)
                ]

            for t in range(NSTEPS):
                hb, qc = PASSES[t // 8]
                kb2 = (t % 8) * 2
                # mid-pass: scores first (keeps ACT fed); at boundaries the
                # new pass's scores wait on the exp pipeline, so run the
                # previous pass's ready attn@v first to keep the PE busy
                if kb2 == 0 and prev_probs:
                    for pkb, ppb in prev_probs:
                        av_mms(accs, prev_pass[0], pkb, ppb)
                    pend_sc = emit_scores(hb, qc, kb2)
                else:
                    pend_sc = emit_scores(hb, qc, kb2)
                    for pkb, ppb in prev_probs:
                        av_mms(accs, prev_pass[0], pkb, ppb)
                if prev_probs and prev_probs[-1][0] == TB - 1:
                    # previous pass complete: drain + defer its epilogue,
                    # then recycle the acc slots for the new pass
                    outs = drain(accs)
                    pend_epi = (prev_pass[0], prev_pass[1], outs, t + 1)
                    accs = new_accs(hb, qc)
                elif accs is None:
                    accs = new_accs(hb, qc)
                # filler work
                for fn in filler[t]:
                    fn()
                # deferred epilogue of the pass before last
                if pend_epi is not None and t >= pend_epi[3]:
                    epi_pe(pend_epi[0], pend_epi[1], pend_epi[2])
                    pend_epi = None
                # exp of this step's scores
                prev_probs = emit_exps(pend_sc)
                prev_pass = (hb, qc)

            # ---- tail: last pass's attn@v drain + epilogue ------------
            if pend_epi is not None:
                epi_pe(pend_epi[0], pend_epi[1], pend_epi[2])
                pend_epi = None
            # head-major: head 0's drain/normalize chain (ACT copies,
            # DVE recip, broadcast matmul, multiply) runs under head 1's
            # attn@v stream
            hb_l, qc_l = prev_pass
            psb_l = ps2.tile([128, 512], F32, tag="psV", bufs=2, name="psbL")
            for hh in range(2):
                for pkb, ppb in prev_probs:
                    av_mms(accs, prev_pass[0], pkb, ppb, heads=(hh,))
                acc = accs[hh]
                den = mi.tile([1, 512], F32, tag="den")
                nc.scalar.copy(den[:, :], acc[64:65, :])
                rec = mi.tile([1, 512], F32, tag="rec")
                nc.vector.reciprocal_approx_fast(rec[:, :], den[:, :])
                row = mi.tile([1, 512], BF16, tag="row")
                nc.vector.tensor_copy(row[:, :], rec[:, :])
                cpy = mi.tile([64, 512], F32, tag="cpy")
                nc.scalar.copy(cpy[:, :], acc[0:64, :])
                nc.tensor.matmul(
                    psb_l[64 * hh : 64 * hh + 64, :],
                    ones_sb[:, :],
                    row[:, :],
                    start=True,
                    stop=True,
                )
                nc.vector.tensor_mul(
                    attnT_sb[
                        64 * hh : 64 * hh + 64,
                        hb_l * NQ + qc_l * 512 : hb_l * NQ + (qc_l + 1) * 512,
                    ],
                    psb_l[64 * hh : 64 * hh + 64, :],
                    cpy[:, :],
                )

            # ---- output projection, qc=1 final terms ------------------
            for ob in range(CB):
                proj_final(ob, 1)

    nc.finalize()
    return nc


_NC_CACHE = []


def _get_nc():
    if not _NC_CACHE:
        _NC_CACHE.append(_build())
    return _NC_CACHE[0]


def kernel(x, w_qkv, w_proj, b_proj):
    x = np.asarray(x, dtype=np.float32)
    w_qkv = np.asarray(w_qkv, dtype=np.float32)
    w_proj = np.asarray(w_proj, dtype=np.float32)
    b_proj = np.asarray(b_proj, dtype=np.float32)

    nc = _get_nc()

    wqkvT = w_qkv.T.astype(ml_dtypes.bfloat16)  # [C, 3C]
    wq3 = np.ascontiguousarray(wqkvT).reshape(CB, 128, 3 * C)  # [ci, p, o]
    wqp = np.concatenate(
        [
            wq3[:, :, o0 : o0 + w].transpose(1, 0, 2).reshape(128, CB * w)
            for o0, w in _WQ_GROUPS
        ],
        axis=1,
    )
    wqp = np.ascontiguousarray(wqp)
    # SBUF images: wproj cols = [ci][o], bias cols = [ci]
    wprojp = np.ascontiguousarray(
        w_proj.T.astype(ml_dtypes.bfloat16).reshape(CB, 128, C)
        .transpose(1, 0, 2)
        .reshape(128, CB * C)
    )
    biasp = np.ascontiguousarray(
        b_proj.astype(np.float32).reshape(CB, 128).T
    )

    in_maps = []
    for core in range(N_CORES):
        b, half = divmod(core, 2)
        # own 1024 query tokens first, then the other half (key order
        # within attention is permutation-invariant)
        mine = x[b, half * NQ : (half + 1) * NQ].T
        other = x[b, (1 - half) * NQ : (2 - half) * NQ].T
        xTc = np.concatenate([mine, other], axis=1).astype(ml_dtypes.bfloat16)
        # pack to the SBUF image: cols = [tch][ci][t]
        xTp = np.ascontiguousarray(
            xTc.reshape(CB, 128, 4, 512).transpose(1, 2, 0, 3).reshape(128, CB * N)
        )
        in_maps.append({"xTp": xTp, "wqp": wqp, "wprojp": wprojp, "biasp": biasp})

    res = run_bass_kernel_spmd(nc, in_maps, core_ids=list(range(N_CORES)))

    out = np.empty((B, N, C), dtype=np.float32)
    for core in range(N_CORES):
        b, half = divmod(core, 2)
        out[b, half * NQ : (half + 1) * NQ, :] = (
            res.results[core]["outT"].astype(np.float32).T
        )
    return out


# revision 30
# speedup vs baseline: 1.0279x; 1.0051x over previous
"""Distributed multi-head attention for TRN2 (8 NeuronCores).

Reference computation (per batch b):
    qkv = x @ w_qkv.T                         # (N, 3C)
    q, k, v = split/reshape to (H, N, D)
    attn = softmax(q @ k.T * D**-0.5)         # per head
    out = (attn @ v) reassembled to (N, C)
    out = out @ w_proj.T + b_proj

Sharding: 8 cores = 4 batches x 2 query-halves. Each core computes k/v
for all 2048 tokens of its batch (duplicated across the 2 cores of a
batch - cheaper than communicating), q for its own 1024 tokens, the
full attention for all 12 heads over its 1024 queries, and the output
projection. No collectives.

Layout strategy (all chosen so no on-chip transposes are needed):
  - host passes x^T and w_qkv^T so projections contract over partitions
  - q,k are produced "d-major" ([head-dim, tokens]) via out^T-form
    matmuls; scores are computed transposed ([keys, queries]) which is
    exactly the layout attn@v consumes as its stationary-side operand
  - softmax needs no max-subtraction (scores ~ N(0,1), fp32 exp range)
  - the denominator rides along as a ones-column appended to v (M=65
    matmuls); normalization uses a K=1 ones-matmul to broadcast 1/denom
    across partitions
  - all matmuls in bf16 (PSUM accumulation is fp32); softmax exp runs
    on the scalar (ACT) engine from PSUM f32, writing bf16 probs

Schedule: one flat software pipeline over 96 score-steps (12 passes x
8 k-block pairs), passes hb-alternating so each pair's k/q projection
filler has a 16-step deadline window (per-step PE filler load ~matches
the ACT exp cadence). At pass boundaries the previous pass's ready
attn@v runs before the new pass's scores so the PE never stalls on the
exp double-buffer. The output projection is decomposed into per-pair
partial sums (SBUF f32, DVE adds) emitted as each pair's attnT half is
finalized, leaving only the last pair's term + fused bias-add on the
tail; the final pass's epilogue is pipelined per-head with its drain
copies on the then-idle ACT engine. Input DMAs stay coarse (the
compiler fans big transfers across all 16 queues) in first-consumption
priority order, and the pre-phase is just 2 projection units.

Self-contained: hardcodes B=4, N=2048, C=768, H=12, D=64.
"""

import numpy as np
import ml_dtypes

import concourse.bass as bass
import concourse.mybir as mybir
from concourse import bacc
from concourse.tile import TileContext
from concourse.bass_utils import run_bass_kernel_spmd

F32 = mybir.dt.float32
BF16 = mybir.dt.bfloat16
EXP = mybir.ActivationFunctionType.Exp

B, N, C = 4, 2048, 768
H, D = 12, 64
SCALE = float(D) ** -0.5  # 0.125
NQ = N // 2  # queries per core: 1024
CB = C // 128  # 6 c-chunks
TB = N // 128  # 16 token blocks
HB = H // 2  # 6 head pairs
VW = H * (D + 1)  # 780: v block width with ones columns

N_CORES = 8

# w_qkv columns, grouped in the order the projection units consume them:
# pair-0 k/q, v (split 512+256 for finer DMA deps), then k/q for pairs
# 1..5. Each group holds its column range for all six 128-row input
# chunks, contiguously.
_WQ_GROUPS = [(C, 128), (0, 128), (2 * C, 512), (2 * C + 512, 256)]
for _ob in range(1, CB):
    _WQ_GROUPS.append((C + _ob * 128, 128))
    _WQ_GROUPS.append((_ob * 128, 128))
_WQ_BASE = {}
_cur = 0
for _o0, _w in _WQ_GROUPS:
    _WQ_BASE[_o0] = (_cur, _w)
    _cur += CB * _w

# pass order: hb-alternating keeps the per-pair k/q filler deadlines
# relaxed (pair hb needed by step 16*hb), balancing PE load per step
PASSES = [(hb, qc) for hb in range(HB) for qc in range(2)]
NSTEPS = len(PASSES) * 8  # 96 score-steps, 2 k-blocks each


def _build():
    nc = bacc.Bacc(None, target_bir_lowering=False)

    # host-packed SBUF images: xTp cols = [tch][ci][t]; wqp cols grouped
    # in consumption order (see _WQ_GROUPS)
    xTp = nc.declare_dram_parameter("xTp", [128, CB * N], BF16, isOutput=False)
    wqp = nc.declare_dram_parameter("wqp", [128, CB * 3 * C], BF16, isOutput=False)
    wprojp = nc.declare_dram_parameter("wprojp", [128, CB * C], BF16, isOutput=False)
    biasp = nc.declare_dram_parameter("biasp", [128, CB], F32, isOutput=False)
    outT = nc.declare_dram_parameter("outT", [C, NQ], BF16, isOutput=True)

    with TileContext(nc) as tc:
        with (
            tc.tile_pool(name="per", bufs=1) as per,
            tc.tile_pool(name="p23", bufs=1) as p23,
            tc.tile_pool(name="hp", bufs=8) as hp,
            tc.tile_pool(name="mi", bufs=3) as mi,
            tc.tile_pool(name="op", bufs=2) as op_pool,
            tc.tile_pool(name="wq", bufs=1) as wq_pool,
            tc.tile_pool(name="xt", bufs=1) as xt_pool,
            tc.tile_pool(name="ps", bufs=2, space="PSUM") as ps2,
        ):
            # ---- persistent tiles -------------------------------------
            qT_sb = per.tile([128, CB * NQ], BF16)  # q^T  [2 heads/blk, 1024]
            kT_sb = per.tile([128, CB * N], BF16)  # k^T  [2 heads/blk, 2048]
            vaug_sb = per.tile([128, TB * VW], BF16)  # v + ones cols
            bias_sb = per.tile([128, CB], F32)
            ones_sb = per.tile([1, 64], BF16)
            attnT_sb = p23.tile([128, CB * NQ], BF16)  # attn out^T
            wproj_sb = p23.tile([128, CB * C], BF16)

            nc.vector.memset(ones_sb[:, :], 1.0)
            # ones columns of vaug: col 64 of each 65-wide head slot
            vaug_ones = vaug_sb[:, :].rearrange(
                "p (t h x) -> p t h x", t=TB, h=H, x=D + 1
            )[:, :, :, D : D + 1]
            nc.vector.memset(vaug_ones, 1.0)

            wqkv_sb = wq_pool.tile([128, CB * 3 * C], BF16)
            # coarse per-chunk x tiles: one big DMA each -- the compiler
            # splits large DMAs row-wise across all 16 queues, so coarse
            # transfers load ~10x faster than per-[128,512] chunk DMAs
            xts = [
                xt_pool.tile([128, CB * 512], BF16, name=f"xt{t}")
                for t in range(4)
            ]

            def _dma_xt(tch, split=1):
                w = CB * 512 // split
                for i in range(split):
                    nc.sync.dma_start(
                        out=xts[tch][:, i * w : (i + 1) * w],
                        in_=xTp[:, tch * CB * 512 + i * w : tch * CB * 512 + (i + 1) * w],
                    )

            def _dma_wq(gi):
                o0, w = _WQ_GROUPS[gi]
                base, _ = _WQ_BASE[o0]
                nc.sync.dma_start(
                    out=wqkv_sb[:, base : base + CB * w],
                    in_=wqp[:, base : base + CB * w],
                )

            # DMA priority order (first-consumption order):
            #   pair0-k + x-chunk0 (pre-phase), pair0-q, x-chunk1,
            #   v weights, x-chunks 2-3, later pairs' k/q, bias/wproj
            _dma_wq(0)
            _dma_xt(0, split=2)
            _dma_wq(1)
            _dma_xt(1)
            _dma_wq(2)
            _dma_wq(3)
            _dma_xt(2)
            _dma_xt(3)
            for gi in range(4, len(_WQ_GROUPS)):
                _dma_wq(gi)
            nc.sync.dma_start(out=bias_sb[:, :], in_=biasp[:, :])
            nc.sync.dma_start(out=wproj_sb[:, :], in_=wprojp[:, :])

            def wq(ci, o0, width):
                base, gw = _WQ_BASE[o0]
                return wqkv_sb[:, base + ci * gw : base + ci * gw + width]

            # ---- projection work units (PE filler) --------------------
            def kq_unit(ob, tch, is_q):
                """one k^T (or q^T) block: out-dims block ob, 512 tokens"""
                t0 = tch * 512
                kind = "q" if is_q else "k"
                psv = ps2.tile(
                    [128, 512], F32, tag="psV", bufs=2, name=f"{kind}{ob}_{tch}"
                )
                for ci in range(CB):
                    nc.tensor.matmul(
                        psv[:, :],
                        wq(ci, (0 if is_q else C) + ob * 128, 128),
                        xts[tch][:, ci * 512 : (ci + 1) * 512],
                        start=(ci == 0),
                        stop=(ci == CB - 1),
                    )
                if is_q:
                    nc.vector.tensor_copy(
                        qT_sb[:, ob * NQ + t0 : ob * NQ + t0 + 512], psv[:, :]
                    )
                else:
                    nc.vector.tensor_copy(
                        kT_sb[:, ob * N + t0 : ob * N + t0 + 512], psv[:, :]
                    )

            def v_unit(t128, o0, w):
                """one v unit: 128 tokens x [o0, o0+w) v-dims, written
                (bf16) into the vaug slot layout"""
                tch, tb = divmod(t128, 4)
                psv = ps2.tile(
                    [128, 512], F32, tag="psV", bufs=2, name=f"v{t128}_{o0}"
                )
                for ci in range(CB):
                    nc.tensor.matmul(
                        psv[:, :w],
                        xts[tch][:, ci * 512 + tb * 128 : ci * 512 + (tb + 1) * 128],
                        wq(ci, 2 * C + o0, w),
                        start=(ci == 0),
                        stop=(ci == CB - 1),
                    )
                nh = w // D
                src = psv[:, :w].rearrange("p (h x) -> p h x", x=D)
                h0 = o0 // D
                base = t128 * VW + h0 * (D + 1)
                dst = vaug_sb[:, base : base + nh * (D + 1)].rearrange(
                    "p (h x) -> p h x", x=D + 1
                )[:, :, :D]
                nc.vector.tensor_copy(dst, src)

            def proj_unit(ob, qc):
                """output projection for out-dims block ob, query half qc
                (contracts all 6 attnT pair-blocks), bias add + DMA out"""
                psp = ps2.tile(
                    [128, 512], F32, tag="psV", bufs=2, name=f"prj{ob}_{qc}"
                )
                for cb in range(CB):
                    nc.tensor.matmul(
                        psp[:, :],
                        wproj_sb[:, cb * C + ob * 128 : cb * C + (ob + 1) * 128],
                        attnT_sb[:, cb * NQ + qc * 512 : cb * NQ + (qc + 1) * 512],
                        start=(cb == 0),
                        stop=(cb == CB - 1),
                    )
                ot = op_pool.tile([128, 512], BF16, tag="out")
                nc.vector.tensor_scalar_add(
                    ot[:, :], psp[:, :], bias_sb[:, ob : ob + 1]
                )
                nc.sync.dma_start(
                    out=outT[ob * 128 : (ob + 1) * 128, qc * 512 : (qc + 1) * 512],
                    in_=ot[:, :],
                )

            # output projection as per-pair partial sums: psq[qc][ob]
            # accumulates wproj_cb^T @ attnT[cb] in SBUF f32 (DVE adds) as
            # each pair's attnT half becomes final; only the last pair's
            # term + bias is on the tail/late critical path
            projp_sb = [
                [p23.tile([128, 512], F32, name=f"pp{qc}_{ob}") for ob in range(CB)]
                for qc in range(2)
            ]

            def proj_partial(cb, ob, qc):
                psp = ps2.tile(
                    [128, 512], F32, tag="psV", bufs=2, name=f"pj{cb}_{ob}_{qc}"
                )
                nc.tensor.matmul(
                    psp[:, :],
                    wproj_sb[:, cb * C + ob * 128 : cb * C + (ob + 1) * 128],
                    attnT_sb[:, cb * NQ + qc * 512 : cb * NQ + (qc + 1) * 512],
                    start=True,
                    stop=True,
                )
                if cb == 0:
                    nc.vector.tensor_copy(projp_sb[qc][ob][:, :], psp[:, :])
                else:
                    nc.vector.tensor_add(
                        projp_sb[qc][ob][:, :], projp_sb[qc][ob][:, :], psp[:, :]
                    )

            def proj_final(ob, qc):
                cb = CB - 1
                psp = ps2.tile(
                    [128, 512], F32, tag="psV", bufs=2, name=f"pjf{ob}_{qc}"
                )
                nc.tensor.matmul(
                    psp[:, :],
                    wproj_sb[:, cb * C + ob * 128 : cb * C + (ob + 1) * 128],
                    attnT_sb[:, cb * NQ + qc * 512 : cb * NQ + (qc + 1) * 512],
                    start=True,
                    stop=True,
                )
                ot = op_pool.tile([128, 512], BF16, tag="out")
                # (psp + bias) + partial, fused on DVE
                nc.vector.scalar_tensor_tensor(
                    ot[:, :],
                    psp[:, :],
                    bias_sb[:, ob : ob + 1],
                    projp_sb[qc][ob][:, :],
                    op0=mybir.AluOpType.add,
                    op1=mybir.AluOpType.add,
                )
                nc.sync.dma_start(
                    out=outT[ob * 128 : (ob + 1) * 128, qc * 512 : (qc + 1) * 512],
                    in_=ot[:, :],
                )

            # ---- filler schedule --------------------------------------
            # filler[t] = list of closures to emit in step t's filler slot
            filler = [[] for _ in range(NSTEPS)]

            def _sched(t, fn):
                filler[t].append(fn)

            # remaining pair-0 k chunks first (their x chunks land before
            # the v weights): k tch needed by scores kb=4*tch (step 2*tch)
            for tch in range(1, 4):
                _sched(tch - 1, (lambda t=tch: kq_unit(0, t, False)))
            _sched(4, (lambda: kq_unit(0, 1, True)))  # q tch1 by step 8
            # v blocks just-in-time for pass 0: block kb in step kb//2
            for kb in range(TB):
                _sched(kb // 2, (lambda kb=kb: v_unit(kb, 0, 512)))
                _sched(kb // 2, (lambda kb=kb: v_unit(kb, 512, 256)))
            # pairs 1-5: k tch j needed by step 16*hb + 2*j, q tch0 by
            # 16*hb, q tch1 by 16*hb + 8; weights arrive ~step 5-8
            for hb in range(1, HB):
                for j in range(4):
                    _sched(16 * hb - 9 + 2 * j,
                           (lambda h=hb, j=j: kq_unit(h, j, False)))
                _sched(16 * hb - 2, (lambda h=hb: kq_unit(h, 0, True)))
                _sched(16 * hb + 4, (lambda h=hb: kq_unit(h, 1, True)))
            # out-proj partials: pair cb's (qc) half is final after pass
            # idx 2*cb+qc's epilogue (~step 16*cb+8*qc+10)
            for cb in range(CB - 1):
                for ob in range(CB):
                    _sched(16 * cb + 11 + ob,
                           (lambda c=cb, o=ob: proj_partial(c, o, 0)))
                    _sched(16 * cb + 19 + ob,
                           (lambda c=cb, o=ob: proj_partial(c, o, 1)))
            # qc=0 finals: pair-5 qc0 epilogue fires at step 89
            for ob in range(CB):
                _sched(90 + ob, (lambda o=ob: proj_final(o, 0)))

            # ---- attention pipeline -----------------------------------
            def epi_pe(hb_, qc_, outs_):
                """PE part of a pass's normalization epilogue. The two
                heads' 1/denom broadcasts go to different column strips of
                one PSUM tile (col tiling) so they run concurrently."""
                psb = ps2.tile(
                    [128, 512], F32, tag="psV", bufs=2,
                    name=f"psb{hb_}_{qc_}",
                )
                for hh_ in range(2):
                    nc.tensor.matmul(
                        psb[64 * hh_ : 64 * hh_ + 64, :],
                        ones_sb[:, :],
                        outs_[hh_][1][:, :],
                        start=True,
                        stop=True,
                    )
                for hh_ in range(2):
                    nc.vector.tensor_mul(
                        attnT_sb[
                            64 * hh_ : 64 * hh_ + 64,
                            hb_ * NQ + qc_ * 512 : hb_ * NQ + (qc_ + 1) * 512,
                        ],
                        psb[64 * hh_ : 64 * hh_ + 64, :],
                        outs_[hh_][0][:, :],
                    )

            def emit_scores(hb, qc, kb2):
                """scores for k-blocks kb2, kb2+1 (both heads) -> psS
                tiles + exp -> probs; returns [(kb, srcs), ...]"""
                q0 = hb * NQ + qc * 512
                out = []
                for kb in (kb2, kb2 + 1):
                    sc = ps2.tile(
                        [128, NQ], F32, tag="psS", bufs=2,
                        name=f"sc{hb}_{qc}_{kb}",
                    )
                    for hh in range(2):
                        p0 = 64 * hh
                        nc.tensor.matmul(
                            sc[:, hh * 512 : (hh + 1) * 512],
                            kT_sb[
                                p0 : p0 + 64,
                                hb * N + kb * 128 : hb * N + (kb + 1) * 128,
                            ],
                            qT_sb[p0 : p0 + 64, q0 : q0 + 512],
                            start=True,
                            stop=True,
                            tile_position=(p0, 0),
                        )
                    out.append((kb, [(sc, 0, NQ)]))
                return out

            def emit_exps(pending_sc):
                out = []
                for kb, srcs in pending_sc:
                    pb = hp.tile([128, NQ], BF16, tag="probs")
                    for src, col, w in srcs:
                        nc.scalar.activation(
                            pb[:, col : col + w], src[:, :w], EXP, scale=SCALE
                        )
                    out.append((kb, pb))
                return out

            def av_mms(accs, hb, pkb, ppb, heads=(0, 1)):
                for hh in heads:
                    vs = pkb * VW + (2 * hb + hh) * (D + 1)
                    nc.tensor.matmul(
                        accs[hh][0:65, :],
                        vaug_sb[:, vs : vs + D + 1],
                        ppb[:, hh * 512 : (hh + 1) * 512],
                        start=(pkb == 0),
                        stop=(pkb == TB - 1),
                    )

            def drain(accs):
                """epilogue: drain accumulators (on the otherwise-idle
                Pool engine) + 1/denominator (DVE)"""
                outs = []
                for hh in range(2):
                    acc = accs[hh]
                    cpy = mi.tile([64, 512], F32, tag="cpy")
                    nc.vector.tensor_copy(cpy[:, :], acc[0:64, :])
                    den = mi.tile([1, 512], F32, tag="den")
                    nc.vector.tensor_copy(den[:, :], acc[64:65, :])
                    rec = mi.tile([1, 512], F32, tag="rec")
                    nc.vector.reciprocal_approx_fast(rec[:, :], den[:, :])
                    row = mi.tile([1, 512], BF16, tag="row")
                    nc.vector.tensor_copy(row[:, :], rec[:, :])
                    outs.append((cpy, row))
                return outs

            # ---- pre-phase: minimal (pair-0 k/q for the first chunk) --
            kq_unit(0, 0, False)
            kq_unit(0, 0, True)

            # ---- the flat pipeline ------------------------------------
            accs = None  # current pass's psA accumulators
            prev_probs = []  # [(kb, probs)] awaiting attn@v
            prev_pass = None  # (hb, qc) owning prev_probs
            pend_epi = None  # (hb, qc, outs, due_step)

            def new_accs(hb, qc):
                return [
                    ps2.tile(
                        [128, 512], F32, tag="psA", bufs=2,
                        name=f"acc{hb}_{qc}_{i}",
                    )
                    for i in range(2)
                ]

            for t in range(NSTEPS):
                hb, qc = PASSES[t // 8]
                kb2 = (t % 8) * 2
                # mid-pass: scores first (keeps ACT fed); at boundaries the
                # new pass's scores wait on the exp pipeline, so run the
                # previous pass's ready attn@v first to keep the PE busy
                if kb2 == 0 and prev_probs:
                    for pkb, ppb in prev_probs:
                        av_mms(accs, prev_pass[0], pkb, ppb)
                    pend_sc = emit_scores(hb, qc, kb2)
                else:
                    pend_sc = emit_scores(hb, qc, kb2)
                    for pkb, ppb in prev_probs:
                        av_mms(accs, prev_pass[0], pkb, ppb)
                if prev_probs and prev_probs[-1][0] == TB - 1:
                    # previous pass complete: drain + defer its epilogue,
                    # then recycle the acc slots for the new pass
                    outs = drain(accs)
                    pend_epi = (prev_pass[0], prev_pass[1], outs, t + 1)
                    accs = new_accs(hb, qc)
                elif accs is None:
                    accs = new_accs(hb, qc)
                # filler work
                for fn in filler[t]:
                    fn()
                # deferred epilogue of the pass before last
                if pend_epi is not None and t >= pend_epi[3]:
                    epi_pe(pend_epi[0], pend_epi[1], pend_epi[2])
                    pend_epi = None
                # exp of this step's scores
                prev_probs = emit_exps(pend_sc)
                prev_pass = (hb, qc)

            # ---- tail: last pass's attn@v drain + epilogue ------------
            if pend_epi is not None:
                epi_pe(pend_epi[0], pend_epi[1], pend_epi[2])
                pend_epi = None
            # head-major: head 0's drain/normalize chain (ACT copies,
            # DVE recip, broadcast matmul, multiply) runs under head 1's
            # attn@v stream
            hb_l, qc_l = prev_pass
            psb_l = ps2.tile([128, 512], F32, tag="psV", bufs=2, name="psbL")
            for hh in range(2):
                for pkb, ppb in prev_probs:
                    av_mms(accs, prev_pass[0], pkb, ppb, heads=(hh,))
                acc = accs[hh]
                den = mi.tile([1, 512], F32, tag="den")
                nc.scalar.copy(den[:, :], acc[64:65, :])
                rec = mi.tile([1, 512], F32, tag="rec")
                nc.vector.reciprocal_approx_fast(rec[:, :], den[:, :])
                row = mi.tile([1, 512], BF16, tag="row")
                nc.vector.tensor_copy(row[:, :], rec[:, :])
                cpy = mi.tile([64, 512], F32, tag="cpy")
                nc.scalar.copy(cpy[:, :], acc[0:64, :])
                nc.tensor.matmul(
                    psb_l[64 * hh : 64 * hh + 64, :],
                    ones_sb[:, :],
                    row[:, :],
                    start=True,
                    stop=True,
                )
                nc.vector.tensor_mul(
                    attnT_sb[
                        64 * hh : 64 * hh + 64,
                        hb_l * NQ + qc_l * 512 : hb_l * NQ + (qc_l + 1) * 512,
                    ],
                    psb_l[64 * hh : 64 * hh + 64, :],
                    cpy[:, :],
                )

            # ---- output projection, qc=1 final terms ------------------
            for ob in range(CB):
                proj_final(ob, 1)

    nc.finalize()
    return nc


_NC_CACHE = []


def _get_nc():
    if not _NC_CACHE:
        _NC_CACHE.append(_build())
    return _NC_CACHE[0]


def kernel(x, w_qkv, w_proj, b_proj):
    x = np.asarray(x, dtype=np.float32)
    w_qkv = np.asarray(w_qkv, dtype=np.float32)
    w_proj = np.asarray(w_proj, dtype=np.float32)
    b_proj = np.asarray(b_proj, dtype=np.float32)

    nc = _get_nc()

    wqkvT = w_qkv.T.astype(ml_dtypes.bfloat16)  # [C, 3C]
    wq3 = np.ascontiguousarray(wqkvT).reshape(CB, 128, 3 * C)  # [ci, p, o]
    wqp = np.concatenate(
        [
            wq3[:, :, o0 : o0 + w].transpose(1, 0, 2).reshape(128, CB * w)
            for o0, w in _WQ_GROUPS
        ],
        axis=1,
    )
    wqp = np.ascontiguousarray(wqp)
    # SBUF images: wproj cols = [ci][o], bias cols = [ci]
    wprojp = np.ascontiguousarray(
        w_proj.T.astype(ml_dtypes.bfloat16).reshape(CB, 128, C)
        .transpose(1, 0, 2)
        .reshape(128, CB * C)
    )
    biasp = np.ascontiguousarray(
        b_proj.astype(np.float32).reshape(CB, 128).T
    )

    in_maps = []
    for core in range(N_CORES):
        b, half = divmod(core, 2)
        # own 1024 query tokens first, then the other half (key order
        # within attention is permutation-invariant)
        mine = x[b, half * NQ : (half + 1) * NQ].T
        other = x[b, (1 - half) * NQ : (2 - half) * NQ].T
        xTc = np.concatenate([mine, other], axis=1).astype(ml_dtypes.bfloat16)
        # pack to the SBUF image: cols = [tch][ci][t]
        xTp = np.ascontiguousarray(
            xTc.reshape(CB, 128, 4, 512).transpose(1, 2, 0, 3).reshape(128, CB * N)
        )
        in_maps.append({"xTp": xTp, "wqp": wqp, "wprojp": wprojp, "biasp": biasp})

    res = run_bass_kernel_spmd(nc, in_maps, core_ids=list(range(N_CORES)))

    out = np.empty((B, N, C), dtype=np.float32)
    for core in range(N_CORES):
        b, half = divmod(core, 2)
        out[b, half * NQ : (half + 1) * NQ, :] = (
            res.results[core]["outT"].astype(np.float32).T
        )
    return out
